# revision 21
# baseline (speedup 1.0000x reference)
"""ContextLSTM Trainium2 kernel.

B=128, T=512, I=1024, H=1024, C=512.  8 NeuronCores, tensor-parallel over
the 4H gate dimension: core d owns h-slice S_d = [128d, 128d+128) and the
four gate row-blocks {i, f, o, g} for that slice (512 gate rows).  Every
timestep each core computes its h chunk, transposes it, and an AllGather
redistributes the full hidden state (bf16) to all cores for the next
step's recurrent matmul.  The input projection x_t @ Wi^T is fused into
the same per-step PSUM accumulation, so it runs inside the AllGather
latency window.  ctx @ Wc^T + (bi+bh+bc) is computed once at the start
(ones-row trick folds the bias into the matmul) and added into PSUM each
step via an identity-matmul.

Matmuls are bf16 (fp32 PSUM accumulation); cell state and elementwise
math stay fp32.
"""

import os
import sys
import numpy as np

try:
    from concourse import bass, bacc, tile, mybir
    from concourse.bass_utils import run_bass_kernel_spmd
except ImportError:
    sys.path.insert(0, "/opt/trn_rl_repo")
    from concourse import bass, bacc, tile, mybir
    from concourse.bass_utils import run_bass_kernel_spmd

import ml_dtypes

BF16 = mybir.dt.bfloat16
F32 = mybir.dt.float32

B, T_FULL, I, H, C = 128, 512, 1024, 1024, 512
NCORES = 8
HL = H // NCORES          # 128 local h dims per core
G = 4 * HL                # 512 local gate columns per core
KI = I // 128             # 8 k-chunks for input projection
KH = H // 128             # 8 k-chunks for recurrence
CP = 640                  # ctx padded (512 ctx + 1 ones + 127 zero)
KC = CP // 128            # 5 k-chunks for ctx projection


def build_nc(T: int):
    nc = bacc.Bacc(
        "TRN2",
        target_bir_lowering=False,
        debug=False,
        num_devices=NCORES,
    )

    xT_d = nc.dram_tensor("xT", [T, 128, I], BF16, kind="ExternalInput")
    wi_d = nc.dram_tensor("wi", [KI, 128, G], BF16, kind="ExternalInput")
    wh_d = nc.dram_tensor("wh", [KH, 128, G], BF16, kind="ExternalInput")
    wcb_d = nc.dram_tensor("wcb", [KC, 128, G], F32, kind="ExternalInput")
    ctxT_d = nc.dram_tensor("ctxT", [KC, 128, B], F32, kind="ExternalInput")
    h0T_d = nc.dram_tensor("h0T", [128, H], BF16, kind="ExternalInput")
    c0l_d = nc.dram_tensor("c0l", [128, HL], F32, kind="ExternalInput")
    ident_d = nc.dram_tensor("ident", [128, 128], BF16, kind="ExternalInput")

    out_d = nc.dram_tensor("out", [128, T, HL], F32, kind="ExternalOutput")
    cf_d = nc.dram_tensor("cf", [128, HL], F32, kind="ExternalOutput")

    rg = [list(range(NCORES))]

    with tile.TileContext(nc) as tc:
        with (
            tc.tile_pool(name="const", bufs=1) as const_pool,
            tc.tile_pool(name="xt", bufs=4) as xt_pool,
            tc.tile_pool(name="hT", bufs=16) as hT_pool,
            tc.tile_pool(name="cst", bufs=2) as c_pool,
            tc.tile_pool(name="act", bufs=2) as act_pool,
            tc.tile_pool(name="small", bufs=3) as small_pool,
            tc.tile_pool(name="gps", bufs=2, space="PSUM") as gates_psum,
            tc.tile_pool(name="tps", bufs=2, space="PSUM") as t_psum,
            tc.tile_pool(name="agi", bufs=2, space="DRAM") as agi_pool,
            tc.tile_pool(name="ago", bufs=2, space="DRAM") as ago_pool,
        ):
            # ---- constants into SBUF ----
            wi_sb = const_pool.tile([128, KI * G], BF16, tag="wi")
            nc.sync.dma_start(
                out=wi_sb[:].rearrange("p (j g) -> p j g", j=KI),
                in_=wi_d[:].rearrange("j p g -> p j g"),
            )
            wh_sb = const_pool.tile([128, KH * G], BF16, tag="wh")
            nc.sync.dma_start(
                out=wh_sb[:].rearrange("p (j g) -> p j g", j=KH),
                in_=wh_d[:].rearrange("j p g -> p j g"),
            )
            wcb_sb = const_pool.tile([128, KC * G], F32, tag="wcb")
            nc.sync.dma_start(
                out=wcb_sb[:].rearrange("p (j g) -> p j g", j=KC),
                in_=wcb_d[:].rearrange("j p g -> p j g"),
            )
            ctxT_sb = const_pool.tile([128, KC * B], F32, tag="ctxT")
            nc.sync.dma_start(
                out=ctxT_sb[:].rearrange("p (j b) -> p j b", j=KC),
                in_=ctxT_d[:].rearrange("j p b -> p j b"),
            )
            ident_sb = const_pool.tile([128, 128], BF16, tag="ident")
            nc.sync.dma_start(out=ident_sb[:], in_=ident_d[:])

            # ---- ctx projection + bias (once) ----
            ctxg_ps = t_psum.tile([128, G], F32, tag="ctxg")
            for j in range(KC):
                nc.tensor.matmul(
                    ctxg_ps[:],
                    ctxT_sb[:, j * B:(j + 1) * B],
                    wcb_sb[:, j * G:(j + 1) * G],
                    start=(j == 0),
                    stop=(j == KC - 1),
                )
            # split ctxg into bf16 hi+lo so it can be injected into the PSUM
            # accumulation via exact identity-matmuls (hi alone would leave a
            # constant offset that compounds over 512 steps)
            ctxg_hi = const_pool.tile([128, G], BF16, tag="ctxg_hi")
            nc.vector.tensor_copy(ctxg_hi[:], ctxg_ps[:])
            ctxg_hi32 = const_pool.tile([128, G], F32, tag="ctxg_hi32")
            nc.vector.tensor_copy(ctxg_hi32[:], ctxg_hi[:])
            ctxg_lo = const_pool.tile([128, G], BF16, tag="ctxg_lo")
            nc.vector.tensor_sub(ctxg_lo[:], ctxg_ps[:], ctxg_hi32[:])

            # ---- initial state ----
            hTA = hT_pool.tile([128, 512], BF16, tag="hTA")
            nc.sync.dma_start(out=hTA[:], in_=h0T_d[:, 0:512])
            hTB = hT_pool.tile([128, 512], BF16, tag="hTB")
            nc.scalar.dma_start(out=hTB[:], in_=h0T_d[:, 512:1024])
            c_prev = c_pool.tile([128, HL], F32, tag="c")
            nc.sync.dma_start(out=c_prev[:], in_=c0l_d[:])

            sig = mybir.ActivationFunctionType.Sigmoid
            tanh = mybir.ActivationFunctionType.Tanh

            for t in range(T):
                xt = xt_pool.tile([128, I], BF16, tag="xt")
                nc.sync.dma_start(out=xt[:], in_=xT_d[t])

                g_ps = gates_psum.tile([128, G], F32, tag="g")
                # ctx/bias term via exact identity-matmuls (runs during the
                # previous step's AllGather window, off the critical path)
                nc.tensor.matmul(
                    g_ps[:], ident_sb[:], ctxg_hi[:], start=True, stop=False
                )
                nc.tensor.matmul(
                    g_ps[:], ident_sb[:], ctxg_lo[:], start=False, stop=False
                )
                # + x_t @ Wi_d.T  (also overlaps the previous AllGather)
                for j in range(KI):
                    nc.tensor.matmul(
                        g_ps[:],
                        xt[:, j * 128:(j + 1) * 128],
                        wi_sb[:, j * G:(j + 1) * G],
                        start=False,
                        stop=False,
                    )
                # + h_{t-1} @ Wh_d.T  (gated on the two half hT DMAs)
                for j in range(KH):
                    src = hTA if j < 4 else hTB
                    nc.tensor.matmul(
                        g_ps[:],
                        src[:, (j % 4) * 128:(j % 4 + 1) * 128],
                        wh_sb[:, j * G:(j + 1) * G],
                        start=False,
                        stop=(j == KH - 1),
                    )

                # gate columns: [i 0:128 | f 128:256 | o 256:384 | g 384:512]
                # ACT reads PSUM directly; order: (i,f) -> g -> o so the cell
                # math can start as early as possible.
                act = act_pool.tile([128, G], F32, tag="act")
                nc.scalar.activation(act[:, 0:2 * HL], g_ps[:, 0:2 * HL], sig)
                nc.scalar.activation(act[:, 3 * HL:G], g_ps[:, 3 * HL:G], tanh)
                nc.scalar.activation(act[:, 2 * HL:3 * HL], g_ps[:, 2 * HL:3 * HL], sig)

                ig = small_pool.tile([128, HL], F32, tag="ig")
                nc.vector.tensor_mul(ig[:], act[:, 0:HL], act[:, 3 * HL:G])
                c_new = c_pool.tile([128, HL], F32, tag="c")
                nc.vector.tensor_mul(c_new[:], act[:, HL:2 * HL], c_prev[:])
                nc.vector.tensor_add(c_new[:], c_new[:], ig[:])

                tch = small_pool.tile([128, HL], F32, tag="tch")
                nc.scalar.activation(tch[:], c_new[:], tanh)

                h_bf = small_pool.tile([128, HL], BF16, tag="hbf")
                nc.vector.tensor_mul(h_bf[:], act[:, 2 * HL:3 * HL], tch[:])

                if t < T - 1:
                    with tc.high_priority():
                        hT_ps = t_psum.tile([128, 128], BF16, tag="hT_ps")
                        nc.tensor.transpose(hT_ps[:], h_bf[:], ident_sb[:])
                        hTl = small_pool.tile([128, 128], BF16, tag="hTl")
                        nc.scalar.activation(
                            hTl[:], hT_ps[:], mybir.ActivationFunctionType.Copy
                        )
                        agin = agi_pool.tile([128, 128], BF16, tag="agin")
                        nc.sync.dma_start(out=agin[:], in_=hTl[:])
                    agout = ago_pool.tile([NCORES * 128, 128], BF16, tag="agout")
                    nc.gpsimd.collective_compute(
                        "AllGather",
                        mybir.AluOpType.bypass,
                        replica_groups=rg,
                        ins=[agin[:].opt()],
                        outs=[agout[:].opt()],
                    )
                    hTA = hT_pool.tile([128, 512], BF16, tag="hTA")
                    nc.sync.dma_start(
                        out=hTA[:].rearrange("p (j b) -> p j b", j=4),
                        in_=agout[0:512, :].rearrange("(j p) b -> p j b", p=128),
                    )
                    hTB = hT_pool.tile([128, 512], BF16, tag="hTB")
                    nc.scalar.dma_start(
                        out=hTB[:].rearrange("p (j b) -> p j b", j=4),
                        in_=agout[512:1024, :].rearrange("(j p) b -> p j b", p=128),
                    )

                # output store (off the critical path)
                h_f32 = small_pool.tile([128, HL], F32, tag="hf32")
                nc.vector.tensor_mul(h_f32[:], act[:, 2 * HL:3 * HL], tch[:])
                nc.sync.dma_start(out=out_d[:, t, :], in_=h_f32[:])
                c_prev = c_new

            nc.sync.dma_start(out=cf_d[:], in_=c_prev[:])

    nc.compile()
    return nc


_NC_CACHE = {}


def _get_nc(T: int):
    if T not in _NC_CACHE:
        _NC_CACHE[T] = build_nc(T)
    return _NC_CACHE[T]


def _bf16(a):
    return np.ascontiguousarray(a).astype(ml_dtypes.bfloat16)


def _prep_inputs(x, h0, c0, ctx, Wi, bi, Wh, bh, Wc, bc, T):
    """Host-side sharding/layout. Returns in_maps (list of dicts, one per core)."""
    x = np.asarray(x, np.float32)
    h0 = np.asarray(h0, np.float32)
    c0 = np.asarray(c0, np.float32)
    ctx = np.asarray(ctx, np.float32)
    Wi = np.asarray(Wi, np.float32)
    Wh = np.asarray(Wh, np.float32)
    Wc = np.asarray(Wc, np.float32)
    bias = (np.asarray(bi, np.float32) + np.asarray(bh, np.float32)
            + np.asarray(bc, np.float32))

    # xT[t, p, j*128+b] = x[b, t, 128j+p]  -> [T, 128, I]
    xv = x[:, :T, :]
    xT = np.ascontiguousarray(
        xv.reshape(B, T, KI, 128).transpose(1, 3, 2, 0)
    ).reshape(T, 128, I)
    xT = _bf16(xT)

    # h0T[p, j*128+b] = h0[b, 128j+p]
    h0T = _bf16(h0.reshape(B, KH, 128).transpose(2, 1, 0).reshape(128, H))

    # ctx padded with ones column (bias row) + zeros
    ctx_aug = np.zeros((B, CP), np.float32)
    ctx_aug[:, :C] = ctx
    ctx_aug[:, C] = 1.0
    ctxT = np.ascontiguousarray(
        ctx_aug.reshape(B, KC, 128).transpose(1, 2, 0)
    )  # [KC, 128, B] fp32

    ident = _bf16(np.eye(128, dtype=np.float32))

    in_maps = []
    for d in range(NCORES):
        sl = slice(HL * d, HL * (d + 1))
        # gate rows, local order [i, f, o, g]
        rows = np.concatenate([
            np.arange(HL * d, HL * (d + 1)),            # i
            H + np.arange(HL * d, HL * (d + 1)),        # f
            3 * H + np.arange(HL * d, HL * (d + 1)),    # o
            2 * H + np.arange(HL * d, HL * (d + 1)),    # g
        ])
        wi_l = _bf16(Wi[rows, :].T.reshape(KI, 128, G))
        wh_l = _bf16(Wh[rows, :].T.reshape(KH, 128, G))
        wcb = np.zeros((CP, G), np.float32)
        wcb[:C, :] = Wc[rows, :].T
        wcb[C, :] = bias[rows]
        wcb_l = np.ascontiguousarray(wcb.reshape(KC, 128, G))
        c0l = np.ascontiguousarray(c0[:, sl])
        in_maps.append({
            "xT": xT, "wi": wi_l, "wh": wh_l, "wcb": wcb_l,
            "ctxT": ctxT, "h0T": h0T, "c0l": c0l, "ident": ident,
        })
    return in_maps


def _ensure_ntff_hook():
    """Best-effort shim: register antenv.axon_hooks so trace=True can
    capture an NTFF profile via libaxon_pjrt's C ABI."""
    import types
    try:
        from antenv.axon_hooks import get_axon_ntff_profile_hook  # noqa: F401
        return True
    except ImportError:
        pass
    try:
        from trn_agent_boot.trn_boot import _ntff_profile_via_ctypes
        hook = _ntff_profile_via_ctypes("/opt/axon/libaxon_pjrt.so")
        mod = types.ModuleType("antenv.axon_hooks")
        mod._hook = hook
        mod.get_axon_ntff_profile_hook = lambda: hook
        mod.set_axon_ntff_profile_hook = lambda h: setattr(mod, "_hook", h)
        import antenv
        sys.modules["antenv.axon_hooks"] = mod
        antenv.axon_hooks = mod
        return hook is not None
    except Exception as e:  # pragma: no cover
        print(f"NTFF hook shim failed: {e}")
        return False


def kernel(x, h0, c0, ctx, Wi, bi, Wh, bh, Wc, bc, _T=None, _trace=None):
    T = T_FULL if _T is None else _T
    if _trace is None:
        _trace = bool(int(os.environ.get("BASS_LSTM_TRACE", "0")))
    if _trace:
        _trace = _ensure_ntff_hook()
    nc = _get_nc(T)
    in_maps = _prep_inputs(x, h0, c0, ctx, Wi, bi, Wh, bh, Wc, bc, T)
    res = run_bass_kernel_spmd(
        nc, in_maps, core_ids=list(range(NCORES)), trace=_trace,
    )
    if _trace and res.exec_time_ns is not None:
        print(f"HW exec time: {res.exec_time_ns} ns")
        kernel.last_exec_time_ns = res.exec_time_ns
    kernel.last_results = res

    outs = [res.results[d]["out"] for d in range(NCORES)]   # [128, T, 128]
    out_full = np.concatenate(outs, axis=2)                 # [B, T, H]
    cf = np.concatenate([res.results[d]["cf"] for d in range(NCORES)], axis=1)
    hf = np.ascontiguousarray(out_full[:, -1, :])
    return out_full, hf, cf


kernel.last_exec_time_ns = None
kernel.last_results = None
